# revision 11
# baseline (speedup 1.0000x reference)
"""Pi0 Gemma-300M action-expert denoise step on 8 Trainium2 NeuronCores.

Strategy: 8-way tensor parallelism — one attention (query) head per core
(Wq/Wo sharded by head), MLP intermediate dim sharded 8x512 (Wg/Wu col,
Wd row), KV cache (single KV head) replicated, AdaRMS scale/gate
projections column-sharded with one AllGather at start, two bf16
AllReduces per layer for the attention and MLP partial sums.

All matmuls run in bf16 (fp32 PSUM accumulation); the residual stream
stays fp32 on-chip. Activations are kept feature-major ([feature-chunk
partitions, token free-dim]) so every projection is
  out_T = lhsT(W natural [K,M]).T @ rhs(X_T [K,N])
with no transposes anywhere on-chip.
"""
import os
import sys
import math

import numpy as np
import ml_dtypes

sys.path.insert(0, "/opt/trn_rl_repo")

# model dims (Pi0 Gemma-300M action expert)
L = int(os.environ.get("BASS_KERNEL_LAYERS", "18"))
H, NH, KVH, HD, FF = 1024, 8, 1, 256, 4096
AH, AD, P, B = 50, 32, 968, 4
T = B * AH                      # 200 tokens
HC = H // 128                   # 8 feature chunks
EPS = 1e-6
ROPE_BASE = 10000.0
SCALING = HD ** -0.5
NC_ = 8                         # cores
FFS = FF // NC_                 # 512 FF rows per core
NMAT = 4 * L + 1                # AdaRMS matrices (+ final)
JT = 8                          # key tiles: 1024 = 968 prefix + 50 new + 6 pad
PFX7 = P - 7 * 128              # 72 prefix rows in tile 7
SFXE = PFX7 + AH                # 122: end of real rows in tile 7
SIM_COMPAT = bool(int(os.environ.get("BASS_KERNEL_SIMCOMPAT", "0")))


def _build_program():
    import concourse.bacc as bacc
    import concourse.mybir as mybir
    import concourse.tile as tile
    from contextlib import ExitStack

    BF16 = mybir.dt.bfloat16
    F32 = mybir.dt.float32
    AF = mybir.ActivationFunctionType
    ALU = mybir.AluOpType

    nc = bacc.Bacc("TRN2", target_bir_lowering=False, num_devices=NC_)

    # ---------------- DRAM I/O ----------------
    d_wqk = nc.dram_tensor("wqk", [L, HC, 128, 512], BF16, kind="ExternalInput")
    d_wv = nc.dram_tensor("wv", [L, HC, 128, 256], BF16, kind="ExternalInput")
    d_wo = nc.dram_tensor("wo", [L, 2, 128, H], BF16, kind="ExternalInput")
    d_wgu = nc.dram_tensor("wgu", [L, HC, 128, 1024], BF16, kind="ExternalInput")
    d_wd = nc.dram_tensor("wd", [L, 4, 128, H], BF16, kind="ExternalInput")
    d_lnw = nc.dram_tensor("lnw", [NMAT, HC, 128, 128], BF16, kind="ExternalInput")
    d_wtime = nc.dram_tensor("wtime", [2, HC, 128, H], BF16, kind="ExternalInput")
    d_emb0 = nc.dram_tensor("emb0", [HC, 128, B], BF16, kind="ExternalInput")
    d_biasv = nc.dram_tensor("biasv", [3, HC, 128], F32, kind="ExternalInput")
    d_wai = nc.dram_tensor("wai", [AD, H], BF16, kind="ExternalInput")
    d_wao = nc.dram_tensor("wao", [HC, 128, AD], BF16, kind="ExternalInput")
    d_aob = nc.dram_tensor("aob", [AD, 1], F32, kind="ExternalInput")
    d_xtt = nc.dram_tensor("xtt", [AD, T], BF16, kind="ExternalInput")
    d_rope = nc.dram_tensor("rope", [128, 4, T], F32, kind="ExternalInput")
    d_kct = nc.dram_tensor("kct", [L, B, 2, 128, P], BF16, kind="ExternalInput")
    d_vc = nc.dram_tensor("vc", [L, B, P, HD], BF16, kind="ExternalInput")
    d_out = nc.dram_tensor("outt", [AD, T], F32, kind="ExternalOutput")
    # AllGather staging for AdaRMS scales
    d_scpart = nc.dram_tensor("scpart", [128, NMAT * 4], F32)
    d_scall = nc.dram_tensor("scall", [NC_, 128, NMAT * 4], F32)

    RG = [list(range(NC_))]

    with tile.TileContext(nc, num_cores=NC_) as tc, ExitStack() as ctx:
        cp = ctx.enter_context(tc.tile_pool(name="const", bufs=1))
        wqk_p = ctx.enter_context(tc.tile_pool(name="wqk", bufs=2))
        wv_p = ctx.enter_context(tc.tile_pool(name="wv", bufs=2))
        wo_p = ctx.enter_context(tc.tile_pool(name="wo", bufs=2))
        wgu_p = ctx.enter_context(tc.tile_pool(name="wgu", bufs=2))
        wd_p = ctx.enter_context(tc.tile_pool(name="wd", bufs=2))
        lnw_p = ctx.enter_context(tc.tile_pool(name="lnw", bufs=3))
        kf_p = ctx.enter_context(tc.tile_pool(name="kf", bufs=3))
        vch_p = ctx.enter_context(tc.tile_pool(name="vch", bufs=3))
        pr_p = ctx.enter_context(tc.tile_pool(name="probs", bufs=3))
        sq_p = ctx.enter_context(tc.tile_pool(name="sq", bufs=2))
        nm_p = ctx.enter_context(tc.tile_pool(name="normed", bufs=2))
        tmp_p = ctx.enter_context(tc.tile_pool(name="tmp", bufs=2))
        ar_p = ctx.enter_context(tc.tile_pool(name="arsb", bufs=2))
        sm_p = ctx.enter_context(tc.tile_pool(name="small", bufs=2))

        # PSUM: 8 banks total.  big(4) + scores(2) + attn(1) + small(1)
        ps_big = ctx.enter_context(tc.tile_pool(name="psBig", bufs=1, space="PSUM"))
        ps_sc = ctx.enter_context(tc.tile_pool(name="psSC", bufs=2, space="PSUM"))
        ps_at = ctx.enter_context(tc.tile_pool(name="psAT", bufs=1, space="PSUM"))
        ps_sm = ctx.enter_context(tc.tile_pool(name="psSM", bufs=1, space="PSUM"))

        dr_p = ctx.enter_context(tc.tile_pool(name="dram", bufs=3, space="DRAM"))

        def big_tile():
            return ps_big.tile([128, 4, 512], F32, tag="big", name="bigps")

        def small_tile():
            return ps_sm.tile([128, 512], F32, tag="small", name="smallps")

        # ---------------- constants ----------------
        ones_row = cp.tile([1, 128], BF16)
        nc.vector.memset(ones_row[:], 1.0)
        ones_col = cp.tile([128, 1], BF16)
        nc.vector.memset(ones_col[:], 1.0)
        eps_t = cp.tile([1, 1], F32)
        nc.vector.memset(eps_t[:], EPS)

        xtt = cp.tile([AD, T], BF16)
        nc.sync.dma_start(xtt[:], d_xtt[:])
        wai = cp.tile([AD, H], BF16)
        nc.sync.dma_start(wai[:], d_wai[:])
        wao = cp.tile([128, HC, AD], BF16)
        nc.sync.dma_start(wao[:], d_wao.rearrange("k p m -> p k m"))
        aob = cp.tile([AD, 1], F32)
        nc.sync.dma_start(aob[:], d_aob[:])
        emb0 = cp.tile([128, HC, B], BF16)
        nc.sync.dma_start(emb0[:], d_emb0.rearrange("k p b -> p k b"))
        biasv = cp.tile([128, 3, HC], F32)
        nc.sync.dma_start(biasv[:], d_biasv.rearrange("v k p -> p v k"))
        rope = cp.tile([128, 4, T], F32)
        nc.sync.dma_start(rope[:], d_rope[:])

        hidden = cp.tile([128, HC, T], F32)
        scales = cp.tile([128, HC, NMAT, B], F32)
        cond = cp.tile([128, HC, B], BF16)
        e1 = cp.tile([128, HC, B], BF16)

        # ---------------- hidden0 = x_t @ act_in_w + b ----------------
        for mc in range(HC):
            h0 = small_tile()
            nc.tensor.matmul(h0[:, 0:T], wai[:, mc * 128:(mc + 1) * 128], xtt[:],
                             start=True, stop=True)
            nc.scalar.activation(hidden[:, mc, :], h0[:, 0:T], AF.Identity,
                                 bias=biasv[:, 2, mc:mc + 1], scale=1.0)

        # ---------------- cond = time MLP ----------------
        for half, (src, dst, bi) in enumerate([(emb0, e1, 0), (e1, cond, 1)]):
            wt = wgu_p.tile([128, HC, 1024], BF16, tag="wgu")
            nc.sync.dma_start(wt[:], d_wtime[half].rearrange("k p m -> p k m"))
            for mc in range(HC):
                pe1 = small_tile()
                for kc in range(HC):
                    nc.tensor.matmul(pe1[:, 0:B], wt[:, kc, mc * 128:(mc + 1) * 128],
                                     src[:, kc, :],
                                     start=(kc == 0), stop=(kc == HC - 1))
                # silu(x) = x * sigmoid(x) (Silu table not in CoreSim)
                sgm = sm_p.tile([128, B], F32, tag="silu_s")
                nc.scalar.activation(sgm[:], pe1[:, 0:B], AF.Sigmoid,
                                     bias=biasv[:, bi, mc:mc + 1], scale=1.0)
                xb = sm_p.tile([128, B], F32, tag="silu_x")
                nc.scalar.activation(xb[:], pe1[:, 0:B], AF.Identity,
                                     bias=biasv[:, bi, mc:mc + 1], scale=1.0)
                nc.vector.tensor_mul(dst[:, mc, :], xb[:], sgm[:])

        # ---------------- AdaRMS scale/gate projections (col-sharded) ----------------
        scp = big_tile()
        scpv = scp[:].rearrange("p a x -> p (a x)")
        for mi in range(NMAT):
            lw = lnw_p.tile([128, HC, 128], BF16)
            nc.sync.dma_start(lw[:], d_lnw[mi].rearrange("k p m -> p k m"))
            for kc in range(HC):
                nc.tensor.matmul(scpv[:, mi * 4:(mi + 1) * 4], lw[:, kc, :],
                                 cond[:, kc, :],
                                 start=(kc == 0), stop=(kc == HC - 1))
        scl = sm_p.tile([128, NMAT * 4], F32, tag="scloc")
        nc.scalar.copy(scl[:], scpv[:, 0:NMAT * 4])
        nc.sync.dma_start(d_scpart[:], scl[:])
        nc.gpsimd.collective_compute(
            "AllGather", ALU.bypass, replica_groups=RG,
            ins=[d_scpart[:].opt()], outs=[d_scall[:].opt()],
        )
        for r in range(NC_):
            nc.sync.dma_start(scales[:, r, :, :], d_scall[r])
        # +1.0 on the scale (not gate) matrices: positions 0,2 mod 4, and the final
        sv = scales[:, :, 0:4 * L, :].rearrange("p k (m f) b -> p k m f b", f=2)
        nc.gpsimd.tensor_scalar_add(sv[:, :, :, 0, :], sv[:, :, :, 0, :], 1.0)
        nc.gpsimd.tensor_scalar_add(scales[:, :, 4 * L, :], scales[:, :, 4 * L, :], 1.0)

        # ---------------- helpers ----------------
        def rms_norm(mi):
            """normed = rms(hidden) * scales[mi]  (scales already hold 1+s)"""
            sq = sq_p.tile([128, HC, T], BF16, tag="sq")
            for kc in range(HC):
                nc.scalar.square(sq[:, kc, :], hidden[:, kc, :])
            ssum = small_tile()
            for kc in range(HC):
                nc.tensor.matmul(ssum[0:1, 0:T], ones_col[:], sq[:, kc, :],
                                 start=(kc == 0), stop=(kc == HC - 1))
            rt = sm_p.tile([1, T], F32, tag="rtmp")
            nc.scalar.activation(rt[:], ssum[0:1, 0:T], AF.Sqrt,
                                 bias=eps_t[:], scale=1.0 / H)
            rr = sm_p.tile([1, T], F32, tag="rrec")
            nc.vector.reciprocal(rr[:], rt[:])
            rb = sm_p.tile([1, T], BF16, tag="rb16")
            nc.scalar.copy(rb[:], rr[:])
            rbc = small_tile()
            nc.tensor.matmul(rbc[:, 0:T], ones_row[:], rb[:], start=True, stop=True)
            normed = nm_p.tile([128, HC, T], BF16, tag="normed")
            for kc in range(HC):
                tmp = tmp_p.tile([128, T], F32, tag="nrm_tmp")
                nc.vector.tensor_mul(tmp[:], hidden[:, kc, :], rbc[:, 0:T])
                ssv = scales[:, kc, mi, :].unsqueeze(2).broadcast_to([128, B, AH])
                nc.vector.tensor_mul(
                    normed[:, kc, :].rearrange("p (b t) -> p b t", b=B),
                    tmp[:].rearrange("p (b t) -> p b t", b=B), ssv)
            return normed

        def residual(ar_sb, mi):
            """hidden += gate[mi] * ar_sb   (on gpsimd, all-SBUF)"""
            for kc in range(HC):
                tmp = tmp_p.tile([128, T], F32, tag="res_tmp")
                gv = scales[:, kc, mi, :].unsqueeze(2).broadcast_to([128, B, AH])
                nc.gpsimd.tensor_mul(
                    tmp[:].rearrange("p (b t) -> p b t", b=B),
                    ar_sb[:, kc, :].rearrange("p (b t) -> p b t", b=B), gv)
                nc.gpsimd.tensor_add(hidden[:, kc, :], hidden[:, kc, :], tmp[:])

        def allreduce(ps_bigtile, mi):
            """cast psum partials to bf16, AllReduce across cores, residual-add."""
            pv = ps_bigtile[:].rearrange("p a (s x) -> p a s x", s=2)[:, :, :, 0:T]
            ari = ar_p.tile([128, 4, 2, T], BF16, tag="ar_in")
            nc.scalar.copy(ari[:], pv)
            di = dr_p.tile([128, HC * T], BF16, tag="dr_in")
            nc.sync.dma_start(di[:], ari[:])
            do = dr_p.tile([128, HC * T], BF16, tag="dr_out")
            nc.gpsimd.collective_compute(
                "AllReduce", ALU.add, replica_groups=RG,
                ins=[di[:].opt()], outs=[do[:].opt()],
            )
            aro = ar_p.tile([128, HC, T], BF16, tag="ar_out")
            nc.sync.dma_start(aro[:], do[:])
            residual(aro, mi)

        # ---------------- transformer layers ----------------
        for l in range(L):
            wqk = wqk_p.tile([128, HC, 512], BF16)
            nc.sync.dma_start(wqk[:], d_wqk[l].rearrange("k p m -> p k m"))
            wv = wv_p.tile([128, HC, 256], BF16)
            nc.sync.dma_start(wv[:], d_wv[l].rearrange("k p m -> p k m"))
            wo = wo_p.tile([128, 2, H], BF16)
            nc.sync.dma_start(wo[:], d_wo[l].rearrange("k p m -> p k m"))
            wgu = wgu_p.tile([128, HC, 1024], BF16, tag="wgu")
            nc.sync.dma_start(wgu[:], d_wgu[l].rearrange("k p m -> p k m"))
            wd = wd_p.tile([128, 4, H], BF16)
            nc.sync.dma_start(wd[:], d_wd[l].rearrange("k p m -> p k m"))

            # ---- AdaRMS 1 + q/k/v projections ----
            normed = rms_norm(4 * l)
            qk = big_tile()   # q[dc] = [:, dc, 0:T]; k[dc] = [:, dc, 256:256+T]
            for m in range(4):
                dst = qk[:, m % 2, (m // 2) * 256:(m // 2) * 256 + T]
                for kc in range(HC):
                    nc.tensor.matmul(dst, wqk[:, kc, m * 128:(m + 1) * 128],
                                     normed[:, kc, :],
                                     start=(kc == 0), stop=(kc == HC - 1))
            vjm = small_tile()   # token-major v: [100, 2, 256] packed in one bank
            for tt in range(2):
                for kc in range(HC):
                    nc.tensor.matmul(vjm[0:100, tt * 256:(tt + 1) * 256],
                                     normed[:, kc, tt * 100:(tt + 1) * 100],
                                     wv[:, kc, :],
                                     start=(kc == 0), stop=(kc == HC - 1))
            vjs = sm_p.tile([100, 2, HD], BF16, tag="vjs")
            nc.scalar.copy(vjs[:], vjm[0:100, 0:512].rearrange("p (a d) -> p a d", a=2))

            # ---- RoPE (q pre-scaled by HD^-0.5 via host tables) ----
            q_rot = tmp_p.tile([128, 2, T], BF16, tag="q_rot")
            k_rot = tmp_p.tile([128, 2, T], BF16, tag="k_rot")
            for (col, dst, ci, si) in ((0, q_rot, 0, 1), (256, k_rot, 2, 3)):
                for dc in range(2):
                    ta = tmp_p.tile([128, T], F32, tag="rope_a")
                    tb = tmp_p.tile([128, T], F32, tag="rope_b")
                    nc.vector.tensor_mul(ta[:], qk[:, 1 - dc, col:col + T],
                                         rope[:, si, :])
                    nc.vector.tensor_mul(tb[:], qk[:, dc, col:col + T],
                                         rope[:, ci, :])
                    if dc == 0:
                        nc.vector.tensor_sub(dst[:, dc, :], tb[:], ta[:])
                    else:
                        nc.vector.tensor_add(dst[:, dc, :], tb[:], ta[:])

            # ---- attention per batch (1 head/core, 1018 keys) ----
            attn = ps_at.tile([128, 2, T], F32, tag="attn")
            for b in range(B):
                kf = kf_p.tile([128, 2, 1024], BF16, tag="kf")
                nc.sync.dma_start(kf[:, :, 0:P],
                                  d_kct[l, b].rearrange("c p j -> p c j"))
                for dc in range(2):
                    nc.vector.tensor_copy(kf[:, dc, P:P + AH],
                                          k_rot[:, dc, b * AH:(b + 1) * AH])
                nc.gpsimd.memset(kf[:, :, P + AH:1024], 0.0)

                vch = vch_p.tile([128, JT, HD], BF16, tag="vch")
                # zero the tile-7 tail first; the DMAs below overwrite [72:122)
                nc.gpsimd.memset(vch[96:128, 7, :], 0.0)
                nc.sync.dma_start(
                    vch[:, 0:7, :],
                    d_vc[l, b, 0:896].rearrange("(t p) d -> p t d", p=128))
                nc.sync.dma_start(vch[0:PFX7, 7, :], d_vc[l, b, 896:P])
                nc.sync.dma_start(vch[PFX7:SFXE, 7, :],
                                  vjs[(b % 2) * AH:(b % 2 + 1) * AH, b // 2, :])

                sc = ps_sc.tile([128, JT, AH], F32, tag="sc")
                for jt in range(JT):
                    for dc in range(2):
                        nc.tensor.matmul(sc[:, jt, :],
                                         kf[:, dc, jt * 128:(jt + 1) * 128],
                                         q_rot[:, dc, b * AH:(b + 1) * AH],
                                         start=(dc == 0), stop=(dc == 1))
                probs = pr_p.tile([128, JT, AH], BF16, tag="probs")
                # zero tile-7 tail first; exp overwrites rows [0:122)
                nc.gpsimd.memset(probs[96:128, 7, :], 0.0)
                nc.scalar.activation(probs[:, 0:7, :], sc[:, 0:7, :], AF.Exp)
                nc.scalar.activation(probs[0:SFXE, 7, :], sc[0:SFXE, 7, :], AF.Exp)

                den = small_tile()
                for jt in range(JT):
                    nc.tensor.matmul(den[0:1, 0:AH], ones_col[:], probs[:, jt, :],
                                     start=(jt == 0), stop=(jt == JT - 1))
                rcp = sm_p.tile([1, AH], F32, tag="rcp")
                nc.vector.reciprocal(rcp[:], den[0:1, 0:AH])
                rcpb = sm_p.tile([1, AH], BF16, tag="rcpb")
                nc.scalar.copy(rcpb[:], rcp[:])
                rbc = small_tile()
                nc.tensor.matmul(rbc[:, 0:AH], ones_row[:], rcpb[:],
                                 start=True, stop=True)
                rbs = sm_p.tile([128, AH], F32, tag="rcp_sb")
                nc.scalar.copy(rbs[:], rbc[:, 0:AH])
                nc.vector.tensor_mul(
                    probs[:], probs[:],
                    rbs[:].unsqueeze(1).broadcast_to([128, JT, AH]))
                for dc in range(2):
                    for jt in range(JT):
                        nc.tensor.matmul(attn[:, dc, b * AH:(b + 1) * AH],
                                         vch[:, jt, dc * 128:(dc + 1) * 128],
                                         probs[:, jt, :],
                                         start=(jt == 0), stop=(jt == JT - 1))

            attn_bf = tmp_p.tile([128, 2, T], BF16, tag="attn_bf")
            nc.scalar.copy(attn_bf[:], attn[:])

            # ---- Wo partials + AllReduce 1 + gated residual ----
            wops = big_tile()
            for m in range(HC):
                for dc in range(2):
                    nc.tensor.matmul(
                        wops[:, m // 2, (m % 2) * 256:(m % 2) * 256 + T],
                        wo[:, dc, m * 128:(m + 1) * 128], attn_bf[:, dc, :],
                        start=(dc == 0), stop=(dc == 1))
            allreduce(wops, 4 * l + 1)

            # ---- AdaRMS 2 + MLP ----
            normed2 = rms_norm(4 * l + 2)
            gu = big_tile()
            for m in range(HC):
                for kc in range(HC):
                    nc.tensor.matmul(
                        gu[:, m // 2, (m % 2) * 256:(m % 2) * 256 + T],
                        wgu[:, kc, m * 128:(m + 1) * 128], normed2[:, kc, :],
                        start=(kc == 0), stop=(kc == HC - 1))
            guv = gu[:].rearrange("p a (s x) -> p a s x", s=2)
            gact = tmp_p.tile([128, 2, 2, T], BF16, tag="gact")
            if SIM_COMPAT:
                # tanh-gelu decomposed from primitives CoreSim implements
                gx = guv[:, 0:2, :, 0:T]
                t_sq = tmp_p.tile([128, 2, 2, T], F32, tag="gelu_sq")
                nc.scalar.square(t_sq[:], gx)
                t_cu = tmp_p.tile([128, 2, 2, T], F32, tag="gelu_cu")
                nc.vector.tensor_mul(t_cu[:], gx, t_sq[:])
                nc.vector.tensor_scalar_mul(t_cu[:], t_cu[:], 0.044715)
                nc.vector.tensor_add(t_cu[:], gx, t_cu[:])
                t_th = tmp_p.tile([128, 2, 2, T], F32, tag="gelu_th")
                nc.scalar.activation(t_th[:], t_cu[:], AF.Tanh,
                                     scale=0.7978845608028654)
                nc.vector.tensor_scalar(t_th[:], t_th[:], 1.0, 0.5,
                                        mybir.AluOpType.add,
                                        mybir.AluOpType.mult)
                nc.vector.tensor_mul(gact[:], gx, t_th[:])
            else:
                nc.scalar.activation(gact[:], guv[:, 0:2, :, 0:T],
                                     AF.Gelu_apprx_tanh)
            gub = tmp_p.tile([128, 4, T], BF16, tag="gub")
            nc.vector.tensor_mul(
                gub[:].rearrange("p (a s) t -> p a s t", a=2),
                guv[:, 2:4, :, 0:T], gact[:])
            wdps = big_tile()
            for m in range(HC):
                for fc in range(4):
                    nc.tensor.matmul(
                        wdps[:, m // 2, (m % 2) * 256:(m % 2) * 256 + T],
                        wd[:, fc, m * 128:(m + 1) * 128], gub[:, fc, :],
                        start=(fc == 0), stop=(fc == 3))
            allreduce(wdps, 4 * l + 3)

        # ---------------- final norm + output projection ----------------
        normf = rms_norm(4 * L)
        outp = small_tile()
        for kc in range(HC):
            nc.tensor.matmul(outp[0:AD, 0:T], wao[:, kc, :], normf[:, kc, :],
                             start=(kc == 0), stop=(kc == HC - 1))
        outs = sm_p.tile([AD, T], F32, tag="out_sb")
        nc.scalar.activation(outs[:], outp[0:AD, 0:T], AF.Identity,
                             bias=aob[:], scale=1.0)
        nc.sync.dma_start(d_out[:], outs[:])

    nc.compile()
    return nc


_NC_CACHE = None


def _get_program():
    global _NC_CACHE
    if _NC_CACHE is None:
        _NC_CACHE = _build_program()
    return _NC_CACHE


def _bf(x):
    return np.ascontiguousarray(x).astype(ml_dtypes.bfloat16)


def _prep_inputs(prefix_keys, prefix_values, prefix_pad_masks, x_t, timestep, params):
    """Build per-core input maps (host-side sharding & layout)."""
    p = {k: np.asarray(v, dtype=np.float32) for k, v in params.items()}
    prefix_keys = np.asarray(prefix_keys, dtype=np.float32)
    prefix_values = np.asarray(prefix_values, dtype=np.float32)
    prefix_pad_masks = np.asarray(prefix_pad_masks)
    x_t = np.asarray(x_t, dtype=np.float32)
    timestep = np.asarray(timestep, dtype=np.float32)

    # replicated tensors
    wv_r = _bf(p["Wv"][:L].reshape(L, HC, 128, KVH * HD))
    wtime = _bf(np.stack([p["t_in_w"], p["t_out_w"]]).reshape(2, HC, 128, H))
    biasv = np.stack([p["t_in_b"], p["t_out_b"], p["act_in_b"]]
                     ).reshape(3, HC, 128).astype(np.float32)
    wai = _bf(p["act_in_w"])                                  # [32, H]
    wao = _bf(p["act_out_w"].reshape(HC, 128, AD))
    aob = p["act_out_b"].reshape(AD, 1).astype(np.float32)
    xtt = _bf(x_t.reshape(T, AD).T)                           # [32, 200]

    # time-embedding sinusoid (host: transcendentals only)
    half = H // 2
    fraction = np.linspace(0.0, 1.0, half, dtype=np.float64)
    period = 0.004 * (4.0 / 0.004) ** fraction
    sf = (2.0 * math.pi) / period
    si = sf[None, :] * timestep.astype(np.float64)[:, None]   # [B, 512]
    emb0 = np.concatenate([np.sin(si), np.cos(si)], axis=1).astype(np.float32)
    emb0t = _bf(emb0.T.reshape(HC, 128, B))

    # RoPE tables (positions from the actual pad masks)
    offsets = prefix_pad_masks.astype(np.int32).sum(axis=-1)  # [B]
    pos = offsets[:, None] + np.arange(AH)[None, :]           # [B, AH]
    inv = 1.0 / (ROPE_BASE ** (np.arange(0, HD, 2, dtype=np.float32) / HD))
    freqs = pos[:, :, None].astype(np.float32) * inv[None, None, :]  # [B, AH, 128]
    cos = np.cos(freqs)
    sin = np.sin(freqs)
    rope_t = np.ascontiguousarray(np.stack([
        (cos * SCALING).reshape(T, 128).T,
        (sin * SCALING).reshape(T, 128).T,
        cos.reshape(T, 128).T,
        sin.reshape(T, 128).T,
    ], axis=1)).astype(np.float32)                            # [128, 4, T]

    # KV cache: [B, L_full, 1, P, HD] -> transposed keys + natural values
    kct = _bf(np.ascontiguousarray(
        prefix_keys[:, :L, 0].transpose(1, 0, 3, 2)).reshape(L, B, 2, 128, P))
    vc = _bf(np.ascontiguousarray(prefix_values[:, :L, 0].transpose(1, 0, 2, 3)))

    in_maps = []
    for c in range(NC_):
        wq_c = p["Wq"][:L, :, c * HD:(c + 1) * HD]            # [L, H, 256]
        wqk_c = _bf(np.concatenate([wq_c, p["Wk"][:L]], axis=2
                                   ).reshape(L, HC, 128, 512))
        wo_c = _bf(p["Wo"][:L, c * HD:(c + 1) * HD, :].reshape(L, 2, 128, H))
        wg_c = p["Wg"][:L, :, c * FFS:(c + 1) * FFS]
        wu_c = p["Wu"][:L, :, c * FFS:(c + 1) * FFS]
        wgu_c = _bf(np.concatenate([wg_c, wu_c], axis=2).reshape(L, HC, 128, 1024))
        wd_c = _bf(p["Wd"][:L, c * FFS:(c + 1) * FFS, :].reshape(L, 4, 128, H))
        lnms = []
        for l in range(L):
            for name in ("ln1_s", "ln1_g", "ln2_s", "ln2_g"):
                lnms.append(p[name][l][:, c * 128:(c + 1) * 128])
        lnms.append(p["lnf_s"][:, c * 128:(c + 1) * 128])
        lnw_c = _bf(np.stack(lnms).reshape(NMAT, HC, 128, 128))
        in_maps.append({
            "wqk": wqk_c, "wv": wv_r, "wo": wo_c, "wgu": wgu_c, "wd": wd_c,
            "lnw": lnw_c, "wtime": wtime, "emb0": emb0t, "biasv": biasv,
            "wai": wai, "wao": wao, "aob": aob, "xtt": xtt, "rope": rope_t,
            "kct": kct, "vc": vc,
        })
    return in_maps


def kernel(prefix_keys, prefix_values, prefix_pad_masks, x_t, timestep, params):
    from concourse.bass_utils import run_bass_kernel_spmd

    nc = _get_program()
    in_maps = _prep_inputs(prefix_keys, prefix_values, prefix_pad_masks,
                           x_t, timestep, params)
    res = run_bass_kernel_spmd(nc, in_maps, core_ids=list(range(NC_)))
    out = res.results[0]["outt"]                              # [32, 200] f32
    return np.ascontiguousarray(out.T).reshape(B, AH, AD).astype(np.float32)
